# revision 23
# baseline (speedup 1.0000x reference)
"""Cosine-similarity retrieval kernel for 8 Trainium2 NeuronCores.

Computes out[n, m] = <x1[n]/||x1[n]||, x2[m]/||x2[m]||> / TEMP for
x1, x2 of shape (8192, 1024) fp32 (output (8192, 8192) fp32).

Sharding: x1 rows data-parallel across the 8 cores (1024-row slabs),
x2 replicated. Each core computes its (1024, 8192) slab of the score
matrix.

Device pipeline (per core), all arithmetic on-device:
  - inputs are uploaded d-major (host transpose only, no host math):
    x1t [d, n_slab], x2t [d, m] fp32; SWDGE DMA casts f32->bf16 on the
    way into SBUF
  - row sums of squares accumulate over k-tiles on DVE (SBUF f32), so
    the PE only runs ONE 512-wide ones-matmul per 512 columns for the
    partition reduction (vs. one per k-tile) -> ~24us less PE work
  - the head (first column block) runs its main matmuls k-OUTER across
    7 PSUM banks so the PE starts real GEMM work as soon as the first
    (x1 k-tile, x2 k-tile) DMA pair lands, instead of idling ~20us and
    tripping the HAM clock throttle
  - x1 is pre-scaled by (1/TEMP)/||x1|| applied in the PSUM->SBUF
    drain (DVE scalar_tensor_tensor: psum * row_scale * col_scale)
  - main GEMM: bf16 matmuls, k-accumulated in PSUM, 512-col chunks,
    one [128,512] PSUM bank per chunk, 7-deep slot pool
"""

import sys

if "/opt/trn_rl_repo" not in sys.path:
    sys.path.insert(0, "/opt/trn_rl_repo")

import numpy as np

TEMP = 0.05
N_CORES = 8

_CACHE = {}


def _build(n_slab, m, d):
    """Build + compile the per-core Bass kernel. Shapes are per-core."""
    from contextlib import ExitStack

    import concourse.mybir as mybir
    import concourse.tile as tile
    from concourse import bacc

    f32 = mybir.dt.float32
    bf16 = mybir.dt.bfloat16
    f16 = mybir.dt.float16
    AF = mybir.ActivationFunctionType
    ALU = mybir.AluOpType

    assert d % 128 == 0 and n_slab % 128 == 0 and m % 1024 == 0
    KT = d // 128          # contraction k-tiles
    NMT = n_slab // 128    # output row tiles (128 rows each)
    CB = 1024              # x2 column block per pipeline stage
    NCB = m // CB
    CHW = 512              # psum chunk width (one PSUM bank)
    NCH = CB // CHW        # chunks per column block (2)

    nc = bacc.Bacc("TRN2", target_bir_lowering=False, debug=False,
                   num_devices=N_CORES)
    x1t = nc.declare_dram_parameter("x1t", [d, n_slab], f32, isOutput=False)
    x2t = nc.declare_dram_parameter("x2t", [d, m], f32, isOutput=False)
    # fp16 output: halves the store-side HBM traffic; the host widens the
    # bits to f32 (exact) after gather
    out = nc.declare_dram_parameter("out", [n_slab, m], f16, isOutput=True)

    x1t_k = x1t.ap().rearrange("(kk p) n -> kk p n", p=128)
    x2t_k = x2t.ap().rearrange("(kk p) mm -> kk p mm", p=128)
    out_ap = out.ap()

    # head covers the first HT (mt, chunk) slots of cb0 k-incrementally
    HT = min(7, NMT * NCH)
    head_slots = [(t // NCH, t % NCH) for t in range(HT)]
    tail_slots = [(t // NCH, t % NCH) for t in range(HT, NMT * NCH)]

    with tile.TileContext(nc) as tc, ExitStack() as ctx:
        resid = ctx.enter_context(tc.tile_pool(name="resid", bufs=1))
        x1n = resid.tile([128, KT, n_slab], bf16)   # bf16 cast of x1t
        srep2 = resid.tile([128, m], f32)           # 1/||x2|| replicated
        n1i = resid.tile([128, NMT], f32)           # (1/TEMP)/||x1|| rowwise
        ones = resid.tile([128, 128], f16)
        nc.vector.memset(ones, 1.0)

        # PSUM: 7 banks cycle through the main [128,512] chunk groups,
        # 1 bank (tag np) serves all norm matmuls.
        ps = ctx.enter_context(tc.tile_pool(name="ps", bufs=HT, space="PSUM"))
        np_ = ctx.enter_context(tc.tile_pool(name="np", bufs=1, space="PSUM"))
        x2p = ctx.enter_context(tc.tile_pool(name="x2p", bufs=5))
        vec = ctx.enter_context(tc.tile_pool(name="vec", bufs=2))
        # 10 square-tile slots: cb1 keeps all 8 of its squared k-tiles
        # alive for the PE-side chunk-1 norm reduction
        sqp = ctx.enter_context(tc.tile_pool(name="sqp", bufs=10))
        ssq2p = ctx.enter_context(tc.tile_pool(name="ssq2", bufs=2))
        ost = ctx.enter_context(tc.tile_pool(name="ost", bufs=4))

        # HAM kick: a burst of dummy matmuls gated only on one memset
        # (first in the DVE queue) so the PE is busy almost from t=0 and
        # the first activity window reads busy -> the clock un-throttles
        # early. The garbage results are never read.
        wsrc = vec.tile([128, 512], bf16, tag="wsrc", name="wsrc", bufs=1)
        nc.vector.memset(wsrc, 0.0)
        warm = np_.tile([128, CHW], f32, tag="np", name="warm")
        for _ in range(8):
            nc.tensor.matmul(warm[:], wsrc[:, :128], wsrc[:],
                             start=True, stop=True)

        # preload both ACT table sets (Square, Sqrt) off the critical path
        dum = vec.tile([128, 1], f32, tag="dum", name="dum", bufs=1)
        nc.vector.memset(dum, 1.0)
        dumo = vec.tile([128, 1], f32, tag="dumo", name="dumo", bufs=1)
        nc.scalar.activation(dumo[:], dum[:], AF.Square)
        nc.scalar.activation(dumo[:], dum[:], AF.Sqrt)

        def x2_norm_tail(cb, ssq2):
            """Per 512-chunk of the accumulated x2 sumsq: ones-matmul
            partition-reduce (replicates the sums across partitions),
            sqrt, reciprocal -> srep2 columns."""
            for i in range(NCH):
                off = cb * CB + i * CHW
                npt = np_.tile([128, CHW], f32, tag="np", name=f"np2_{cb}_{i}")
                nc.tensor.matmul(npt[:], ones[:], ssq2[:, i * CHW:(i + 1) * CHW],
                                 start=True, stop=True)
                tmp = vec.tile([128, CHW], f32, tag="vtmp", name="b_tmp")
                nc.scalar.activation(tmp[:], npt[:], AF.Sqrt)
                nc.vector.reciprocal_approx_fast(out=srep2[:, off:off + CHW],
                                                 in_=tmp[:])

        def accum_sq(sq, acc, k, eng):
            """acc = squares accumulated over k. In steady state this
            runs on GPSIMD (eng=nc.gpsimd): the DVE is loaded with the
            PSUM drains there, and a DVE backlog stalls the PE on its
            PSUM slots. In the head the DVE is free, and GpSimd must
            not stall on the ACT squares between its dma_start issues,
            so the head passes eng=nc.vector."""
            if k == 0:
                eng.tensor_copy(out=acc[:], in_=sq[:])
            else:
                eng.tensor_tensor(
                    out=acc[:], in0=sq[:], in1=acc[:], op=ALU.add)

        def drain(pst, mt, i, cb, ot):
            """psum -> (row scale, col scale) -> ot columns (f16)."""
            nc.vector.scalar_tensor_tensor(
                out=ot[:, i * CHW:(i + 1) * CHW], in0=pst[:],
                scalar=n1i[:, mt:mt + 1],
                in1=srep2[:, cb * CB + i * CHW:cb * CB + (i + 1) * CHW],
                op0=ALU.mult, op1=ALU.mult)

        def store(mt, cb, ot):
            nc.sync.dma_start(
                out=out_ap[mt * 128:(mt + 1) * 128, cb * CB:(cb + 1) * CB],
                in_=ot[:])

        # ---- head: k-incremental streaming of x1 + x2 cb0 ----
        # Per k-tile pair (x1-k, x2cb0-k) the PE gets HT main matmuls
        # (k-outer accumulation into 7 banks) so it keeps pace with the
        # DMA stream from ~3us in.
        x2cb0 = x2p.tile([128, KT, CB], bf16, tag="x2cb", name="x2cb0")
        ssq2_0 = ssq2p.tile([128, CB], f16, tag="ssq2", name="ssq2_0")
        np1 = np_.tile([128, CHW], f32, tag="np", name="np1")
        with tc.tile_pool(name="a_sq", bufs=2) as a_sq:
            hps = [ps.tile([128, CHW], f32, tag="ps", name=f"hps{t}")
                   for t in range(HT)]
            for k in range(KT):
                nc.gpsimd.dma_start(out=x1n[:, k, :], in_=x1t_k[k])
                # a pacing matmul keyed directly on the x1-k DMA (before
                # the square's ACT latency) — writes a scratch column
                # range of the norm bank, start=False so it never clears
                # the open np1 accumulation group's bits
                nc.tensor.matmul(np1[:, 8:264], x1n[:, k, 0:128],
                                 wsrc[:, 0:256], start=False, stop=False,
                                 skip_group_check=True)
                nc.gpsimd.dma_start(out=x2cb0[:, k, :], in_=x2t_k[k][:, 0:CB])
                # x1 norms accumulate on the PE: per (k, mt) one width-1
                # matmul with the squared k-tile as stationary. These 8
                # matmuls per k-tile are paced by the x1-k DMA, keeping
                # the PE activity window busy while x2-k streams in.
                sq = a_sq.tile([128, n_slab], f16, tag="a_sq", name="a_sqt")
                nc.scalar.activation(sq[:], x1n[:, k, :], AF.Square)
                for mt in range(NMT):
                    nc.tensor.matmul(np1[:, mt:mt + 1],
                                     sq[:, mt * 128:(mt + 1) * 128],
                                     ones[:, 0:1],
                                     start=(k == 0 and mt == 0),
                                     stop=(k == KT - 1 and mt == NMT - 1),
                                     skip_group_check=True)
                sqb = sqp.tile([128, CB], f16, tag="b_sq", name="b_sqt0")
                nc.scalar.activation(sqb[:], x2cb0[:, k, :], AF.Square)
                accum_sq(sqb, ssq2_0, k, nc.vector)
                for t, (mt, i) in enumerate(head_slots):
                    nc.tensor.matmul(
                        hps[t][:],
                        x1n[:, k, mt * 128:(mt + 1) * 128],
                        x2cb0[:, k, i * CHW:(i + 1) * CHW],
                        start=(k == 0), stop=(k == KT - 1))

        tmp8 = vec.tile([128, NMT], f32, tag="tmp8", name="tmp8", bufs=1)
        # sqrt(nsq * TEMP^2) = ||x1|| * TEMP ; reciprocal -> (1/TEMP)/||x1||
        nc.scalar.activation(tmp8[:], np1[:, 0:NMT], AF.Sqrt,
                             scale=float(TEMP * TEMP))
        nc.vector.reciprocal_approx_fast(out=n1i[:], in_=tmp8[:])
        x2_norm_tail(0, ssq2_0)

        # drain the 7 head slots; mt0-2 pair into full [128,1024] stores
        hot = {}
        for t, (mt, i) in enumerate(head_slots):
            if mt not in hot:
                hot[mt] = ost.tile([128, CB], f16, tag="c_ot", name=f"hot{mt}")
            drain(hps[t], mt, i, 0, hot[mt])
            if i == NCH - 1:
                store(mt, 0, hot[mt])

        # ---- steady state ----
        def main_group(mt, i, cb, x2cb, ot):
            pst = ps.tile([128, CHW], f32, tag="ps", name=f"ps_{cb}_{mt}_{i}")
            for k in range(KT):
                nc.tensor.matmul(
                    pst[:],
                    x1n[:, k, mt * 128:(mt + 1) * 128],
                    x2cb[:, k, i * CHW:(i + 1) * CHW],
                    start=(k == 0), stop=(k == KT - 1))
            drain(pst, mt, i, cb, ot)

        # finish cb0's remaining slots (x2cb0 fully resident)
        for (mt, i) in tail_slots:
            if mt in hot:
                ot = hot[mt]          # (3,1): complete the head's mt3 store
            elif i == 0:
                ot = ost.tile([128, CB], f16, tag="c_ot", name=f"ot0_{mt}")
                hot[mt] = ot
            else:
                ot = hot[mt]
            main_group(mt, i, 0, x2cb0, ot)
            if i == NCH - 1:
                store(mt, 0, ot)

        for cb in range(1, NCB):
            x2cb = x2p.tile([128, KT, CB], bf16, tag="x2cb", name=f"x2cb{cb}")
            ssq2 = ssq2p.tile([128, CB], f16, tag="ssq2", name=f"ssq2_{cb}")
            # all DMA issues first: the accum ops below wait on the ACT
            # squares, and must not block later dma_starts in the GpSimd
            # queue
            for k in range(KT):
                nc.gpsimd.dma_start(out=x2cb[:, k, :],
                                    in_=x2t_k[k][:, cb * CB:(cb + 1) * CB])
            for k in range(KT):
                sqb = sqp.tile([128, CB], f16, tag="b_sq",
                               name=f"b_sq{cb}_{k}")
                nc.scalar.activation(sqb[:], x2cb[:, k, :], AF.Square)
                accum_sq(sqb, ssq2, k, nc.gpsimd)
            x2_norm_tail(cb, ssq2)
            for mt in range(NMT):
                ot = ost.tile([128, CB], f16, tag="c_ot", name=f"ot{cb}_{mt}")
                for i in range(NCH):
                    main_group(mt, i, cb, x2cb, ot)
                store(mt, cb, ot)

    nc.compile()
    return nc


def _get_nc(n_slab, m, d):
    key = (n_slab, m, d)
    if key not in _CACHE:
        _CACHE[key] = _build(n_slab, m, d)
    return _CACHE[key]


def _in_maps(x1, x2, n_slab):
    x1t = np.ascontiguousarray(x1.T)  # [d, n]
    x2t = np.ascontiguousarray(x2.T)  # [d, m]
    return [
        {"x1t": np.ascontiguousarray(x1t[:, i * n_slab:(i + 1) * n_slab]),
         "x2t": x2t}
        for i in range(N_CORES)
    ]


def kernel(x1, x2):
    from concourse.bass_utils import run_bass_kernel_spmd

    x1 = np.asarray(x1, dtype=np.float32)
    x2 = np.asarray(x2, dtype=np.float32)
    n, d = x1.shape
    m, d2 = x2.shape
    assert d == d2 and n % N_CORES == 0
    n_slab = n // N_CORES

    nc = _get_nc(n_slab, m, d)
    res = run_bass_kernel_spmd(nc, _in_maps(x1, x2, n_slab),
                               core_ids=list(range(N_CORES)))
    return np.concatenate([res.results[i]["out"] for i in range(N_CORES)],
                          axis=0).astype(np.float32)


if __name__ == "__main__":
    # small-shape self test
    rng = np.random.default_rng(0)
    n, m, d = 1024, 2048, 256
    x1 = rng.standard_normal((n, d), dtype=np.float32)
    x2 = rng.standard_normal((m, d), dtype=np.float32)
    got = kernel(x1, x2)
    x1n = x1 / np.linalg.norm(x1, axis=1, keepdims=True)
    x2n = x2 / np.linalg.norm(x2, axis=1, keepdims=True)
    want = (x1n @ x2n.T) / TEMP
    rel = np.linalg.norm(got - want) / np.linalg.norm(want)
    print("rel l2 err:", rel)
    print("max abs err:", np.abs(got - want).max(), "scale:", np.abs(want).max())
